# revision 9
# baseline (speedup 1.0000x reference)
# Cross-attention with LoRA adapters + IP-adapter branch, on 8 TRN2 NeuronCores.
# Data-parallel over batch: core b computes batch element b end-to-end.
#
# Per-core math (all matmuls in bf16 operands, fp32 PSUM accumulation):
#   qT  = Wq^T x^T + q_up^T (q_down^T x^T)          [ID, N]   (id-major)
#   kTc = [Wk^T ctxT + lora | Wk_ipa^T ctxT_ipa]    [ID, 81]
#   vvc = block-diag([v | 1], [v_ipa | 1])          [81, H, 130]
#   sT_h = kTc_h^T qT_h ; pT = exp(SCALE * sT)      [81, Tchunk]
#   o_h  = pT_h^T @ vvc_h -> [tok, num_t|d_t|num_i|d_i]; out = num_t/d_t + num_i/d_i
#   y    = outT^T Wo + (outT^T o_down) o_up + bo    (bias+lora ride one matmul)
import numpy as np
from contextlib import ExitStack

import concourse.bass as bass
import concourse.mybir as mybir
import concourse.tile as tile
from concourse import bacc
from concourse.bass_utils import run_bass_kernel_spmd
from concourse.masks import make_identity

F32 = mybir.dt.float32
BF16 = mybir.dt.bfloat16
AF = mybir.ActivationFunctionType
ALU = mybir.AluOpType

B, N, QD, CD, ID = 8, 4096, 1024, 768, 1024
H, DH, R = 16, 64, 32
CTXT, CTXI, CTX = 77, 4, 81
MP = 96           # text block padded to 96 partitions (zero K-cols)
CP = 100          # padded ctx: 0:77 text, 77:96 zeros, 96:100 ipa
SCALE = DH ** -0.5
LORA_W = 1.0
IPA_SCALE = 1.0
TCH = 512
NCH = N // TCH
SUB = 128
NSUB = TCH // SUB
KQ = QD // 128   # 8
KC = CD // 128   # 6
NID = ID // 128  # 8

IN_SPECS = [
    ("x", [N, QD]), ("context", [CTX, CD]),
    ("Wq", [QD, ID]), ("Wk", [CD, ID]), ("Wv", [CD, ID]), ("Wo", [ID, QD]),
    ("bo", [QD]),
    ("q_down", [QD, R]), ("q_up", [R, ID]), ("q_alpha", [1]),
    ("k_down", [CD, R]), ("k_up", [R, ID]), ("k_alpha", [1]),
    ("v_down", [CD, R]), ("v_up", [R, ID]), ("v_alpha", [1]),
    ("o_down", [ID, R]), ("o_up", [R, QD]), ("o_alpha", [1]),
    ("Wk_ipa", [CD, ID]), ("Wv_ipa", [CD, ID]),
]


def _load_alpha(nc, pool, psum_pool, ones_row, d, name):
    """alpha[1] DRAM -> [128,1] f32 sbuf holding alpha * LORA_W / R on every partition.

    Broadcast across partitions via PE: ones[1,128]^T @ alpha[1,1]."""
    a1 = pool.tile([1, 1], F32, tag=f"a1_{name}")
    nc.sync.dma_start(a1, d[name][None, :])
    pa = psum_pool.tile([128, 1], F32, tag="psa")
    nc.tensor.matmul(pa, ones_row, a1, start=True, stop=True)
    a = pool.tile([128, 1], F32, tag=f"a_{name}")
    nc.vector.tensor_scalar_mul(a, pa, LORA_W / R)
    return a


def _cast_weight(nc, pool, wpool, d, name, kchunks, ncols, scale_ap=None):
    """[kchunks*128, ncols] f32 DRAM -> [128, kchunks, ncols] bf16 sbuf."""
    dst = wpool.tile([128, kchunks, ncols], BF16, tag=f"w_{name}")
    src = d[name].rearrange("(ko ki) n -> ki ko n", ki=128)
    for k in range(kchunks):
        st = pool.tile([128, ncols], F32, tag="wstage")
        nc.sync.dma_start(st, src[:, k, :])
        if scale_ap is not None:
            nc.vector.tensor_scalar_mul(dst[:, k, :], st, scale_ap)
        else:
            nc.scalar.activation(dst[:, k, :], st, AF.Copy)
    return dst


def _build(tc, nc, d, out_ap):
    ctx = ExitStack()
    with ctx:
        wpool = ctx.enter_context(tc.tile_pool(name="w", bufs=1))
        ident = wpool.tile([128, 128], BF16, tag="ident")
        make_identity(nc, ident)

        # ---- phase 0: weights, context projections ----
        with tc.tile_pool(name="setup", bufs=3) as sp, \
             tc.tile_pool(name="psa", bufs=1, space="PSUM") as psa:
            ones_row = sp.tile([1, 128], F32, tag="ones_row")
            nc.vector.memset(ones_row, 1.0)
            a_q = _load_alpha(nc, sp, psa, ones_row, d, "q_alpha")
            a_k = _load_alpha(nc, sp, psa, ones_row, d, "k_alpha")
            a_v = _load_alpha(nc, sp, psa, ones_row, d, "v_alpha")
            a_o = _load_alpha(nc, sp, psa, ones_row, d, "o_alpha")

            wq = _cast_weight(nc, sp, wpool, d, "Wq", KQ, ID)
            wo = _cast_weight(nc, sp, wpool, d, "Wo", NID, QD)
            qd_ = _cast_weight(nc, sp, wpool, d, "q_down", KQ, R, scale_ap=a_q)
            od_ = _cast_weight(nc, sp, wpool, d, "o_down", NID, R, scale_ap=a_o)

            # q_up [R, ID] bf16
            q_up = wpool.tile([R, ID], BF16, tag="q_up")
            st = sp.tile([R, ID], F32, tag="upstage")
            nc.sync.dma_start(st, d["q_up"])
            nc.scalar.activation(q_up, st, AF.Copy)
            # o_up_aug [R+1, QD]: rows 0..R-1 = o_up, row R = bo
            o_up = wpool.tile([R + 1, QD], BF16, tag="o_up")
            st = sp.tile([R, ID], F32, tag="upstage")
            nc.sync.dma_start(st, d["o_up"])
            nc.scalar.activation(o_up[0:R, :], st, AF.Copy)
            stb = sp.tile([1, QD], F32, tag="bostage")
            nc.sync.dma_start(stb, d["bo"][None, :])
            nc.scalar.activation(o_up[R:R + 1, :], stb, AF.Copy)

            # context -> ctxT [128, KC, 81] bf16 (feature-major)
            ctxf = sp.tile([CTX, CD], F32, tag="ctxf")
            nc.sync.dma_start(ctxf, d["context"])
            ctxb = sp.tile([CTX, CD], BF16, tag="ctxb")
            nc.vector.tensor_copy(ctxb, ctxf)
            ctxT = wpool.tile([128, KC, CTX], BF16, tag="ctxT")
            with tc.tile_pool(name="ptr0", bufs=2, space="PSUM") as ptr:
                for c in range(KC):
                    pt = ptr.tile([128, CTX], BF16, tag="ptr0")
                    nc.tensor.transpose(pt, ctxb[:, c * 128:(c + 1) * 128], ident[0:CTX, 0:CTX])
                    nc.vector.tensor_copy(ctxT[:, c, :], pt)

            # K/V weights (scoped; freed after setup)
            with tc.tile_pool(name="kvw", bufs=1) as kvp, \
                 tc.tile_pool(name="pskv", bufs=4, space="PSUM") as ps:
                wk = _cast_weight(nc, kvp, kvp, d, "Wk", KC, ID)
                wv = _cast_weight(nc, kvp, kvp, d, "Wv", KC, ID)
                wki = _cast_weight(nc, kvp, kvp, d, "Wk_ipa", KC, ID)
                wvi = _cast_weight(nc, kvp, kvp, d, "Wv_ipa", KC, ID)
                kd_ = _cast_weight(nc, kvp, kvp, d, "k_down", KC, R, scale_ap=a_k)
                vd_ = _cast_weight(nc, kvp, kvp, d, "v_down", KC, R, scale_ap=a_v)
                for nm in ("k_up", "v_up"):
                    st = sp.tile([R, ID], F32, tag="upstage")
                    nc.sync.dma_start(st, d[nm])
                    up = kvp.tile([R, ID], BF16, tag=f"w_{nm}")
                    nc.scalar.activation(up, st, AF.Copy)
                    if nm == "k_up":
                        k_up = up
                    else:
                        v_up = up

                # lora down-projections of text context: [R, 77]
                l1k = kvp.tile([R, CTXT], BF16, tag="l1k")
                l1v = kvp.tile([R, CTXT], BF16, tag="l1v")
                for dst, dn in ((l1k, kd_), (l1v, vd_)):
                    pk = ps.tile([R, CTXT], F32, tag="pskv")
                    for c in range(KC):
                        nc.tensor.matmul(pk, dn[:, c, :], ctxT[:, c, 0:CTXT],
                                         start=(c == 0), stop=(c == KC - 1))
                    nc.vector.tensor_copy(dst, pk)

                # kTc [128, NID, CP]: cols 0:77 text K^T (with lora), 77:96 zero pad,
                # 96:100 ipa K^T. Pad cols give sT rows = 0 -> exp = 1 -> x0 in vvc.
                kTc = wpool.tile([128, NID, CP], BF16, tag="kTc")
                nc.vector.memset(kTc, 0.0)
                for i in range(NID):
                    pk = ps.tile([128, CP], F32, tag="pskv")
                    isl = slice(i * 128, (i + 1) * 128)
                    for c in range(KC):
                        nc.tensor.matmul(pk[:, 0:CTXT], wk[:, c, isl], ctxT[:, c, 0:CTXT],
                                         start=(c == 0), stop=False)
                    nc.tensor.matmul(pk[:, 0:CTXT], k_up[:, isl], l1k, start=False, stop=True)
                    for c in range(KC):
                        nc.tensor.matmul(pk[:, MP:CP], wki[:, c, isl], ctxT[:, c, CTXT:CTX],
                                         start=(c == 0), stop=(c == KC - 1))
                    nc.vector.tensor_copy(kTc[:, i, 0:CTXT], pk[:, 0:CTXT])
                    nc.vector.tensor_copy(kTc[:, i, MP:CP], pk[:, MP:CP])

                # vvc [CP, H, 130] block-diag: [v_h | 1 | 0] over text rows 0:77,
                # zeros 77:96, [0 | v_ipa_h | 1] over ipa rows 96:100
                vvc = wpool.tile([CP, H, 130], BF16, tag="vvc")
                nc.vector.memset(vvc, 0.0)
                nc.vector.memset(vvc[0:CTXT, :, 64:65], 1.0)
                nc.vector.memset(vvc[MP:CP, :, 129:130], 1.0)
                for half in range(2):
                    hsl = slice(half * 512, (half + 1) * 512)
                    pv = ps.tile([CTXT, 512], F32, tag="pskv")
                    for c in range(KC):
                        nc.tensor.matmul(pv, ctxT[:, c, 0:CTXT], wv[:, c, hsl],
                                         start=(c == 0), stop=False)
                    nc.tensor.matmul(pv, l1v, v_up[:, hsl], start=False, stop=True)
                    nc.vector.tensor_copy(
                        vvc[0:CTXT, half * 8:(half + 1) * 8, 0:64],
                        pv.rearrange("p (h e) -> p h e", e=64))
                    pvi = ps.tile([CTXI, 512], F32, tag="pskv")
                    for c in range(KC):
                        nc.tensor.matmul(pvi, ctxT[:, c, CTXT:CTX], wvi[:, c, hsl],
                                         start=(c == 0), stop=(c == KC - 1))
                    if IPA_SCALE != 1.0:
                        nc.vector.tensor_scalar_mul(
                            vvc[MP:CP, half * 8:(half + 1) * 8, 65:129],
                            pvi.rearrange("p (h e) -> p h e", e=64), IPA_SCALE)
                    else:
                        nc.vector.tensor_copy(
                            vvc[MP:CP, half * 8:(half + 1) * 8, 65:129],
                            pvi.rearrange("p (h e) -> p h e", e=64))

        # ---- phase 1: stream token chunks ----
        xfp = ctx.enter_context(tc.tile_pool(name="xf", bufs=3))
        xbp = ctx.enter_context(tc.tile_pool(name="xb", bufs=2))
        xtp = ctx.enter_context(tc.tile_pool(name="xt", bufs=2))
        qtp = ctx.enter_context(tc.tile_pool(name="qt", bufs=2))
        l1p = ctx.enter_context(tc.tile_pool(name="l1", bufs=2))
        ptp = ctx.enter_context(tc.tile_pool(name="pt", bufs=H + 2))
        stp = ctx.enter_context(tc.tile_pool(name="st", bufs=2))
        dnp = ctx.enter_context(tc.tile_pool(name="dn", bufs=2))
        obp = ctx.enter_context(tc.tile_pool(name="ob", bufs=2))
        otp = ctx.enter_context(tc.tile_pool(name="ot", bufs=2))
        ysp = ctx.enter_context(tc.tile_pool(name="ys", bufs=3))
        psb = ctx.enter_context(tc.tile_pool(name="psb", bufs=4, space="PSUM"))
        pst = ctx.enter_context(tc.tile_pool(name="pst", bufs=2, space="PSUM"))
        pso = ctx.enter_context(tc.tile_pool(name="pso", bufs=2, space="PSUM"))

        for t in range(NCH):
            # x chunk -> bf16 -> xT [128, KQ, TCH] (feature-major) via PE transpose
            xT = xtp.tile([128, KQ, TCH], BF16, tag="xT")
            for s in range(NSUB):
                rows = slice(t * TCH + s * SUB, t * TCH + (s + 1) * SUB)
                xf = xfp.tile([128, QD], F32, tag="xf")
                nc.sync.dma_start(xf, d["x"][rows, :])
                xb = xbp.tile([128, QD], BF16, tag="xb")
                nc.vector.tensor_copy(xb, xf)
                for f in range(KQ):
                    pt = pst.tile([128, 128], BF16, tag="pst")
                    nc.tensor.transpose(pt, xb[:, f * 128:(f + 1) * 128], ident)
                    nc.vector.tensor_copy(xT[:, f, s * SUB:(s + 1) * SUB], pt)

            # l1qT [R, TCH] = q_down^T xT (alpha folded into q_down)
            pl = psb.tile([R, TCH], F32, tag="psb")
            for k in range(KQ):
                nc.tensor.matmul(pl, qd_[:, k, :], xT[:, k, :], start=(k == 0), stop=(k == KQ - 1))
            l1q = l1p.tile([R, TCH], BF16, tag="l1q")
            nc.scalar.activation(l1q, pl, AF.Copy)

            # qT [128, NID, TCH] = Wq^T xT + q_up^T l1qT
            qT = qtp.tile([128, NID, TCH], BF16, tag="qT")
            for i in range(NID):
                pq = psb.tile([128, TCH], F32, tag="psb")
                isl = slice(i * 128, (i + 1) * 128)
                for k in range(KQ):
                    nc.tensor.matmul(pq, wq[:, k, isl], xT[:, k, :], start=(k == 0), stop=False)
                nc.tensor.matmul(pq, q_up[:, isl], l1q, start=False, stop=True)
                nc.scalar.activation(qT[:, i, :], pq, AF.Copy)

            # attention: sT_h [CP, TCH] -> pT = exp(SCALE * sT)
            pTs = []
            for h in range(H):
                hp = slice((h % 2) * 64, (h % 2) * 64 + 64)
                hc = h // 2
                psT = psb.tile([CP, TCH], F32, tag="psb")
                nc.tensor.matmul(psT, kTc[hp, hc, :], qT[hp, hc, :], start=True, stop=True)
                pT = ptp.tile([CP, TCH], BF16, tag="pT")
                nc.scalar.activation(pT, psT, AF.Exp, scale=SCALE)
                pTs.append(pT)

            # o-matmuls + combine per 128-token subchunk
            outb_s = []
            for s in range(NSUB):
                ssl = slice(s * SUB, (s + 1) * SUB)
                stg = stp.tile([128, H, 129], BF16, tag="stg")
                den = dnp.tile([128, H, 2], F32, tag="den")
                for h in range(H):
                    po = pso.tile([128, 130], F32, tag="pso")
                    nc.tensor.matmul(po, pTs[h][:, ssl], vvc[:, h, :], start=True, stop=True)
                    nc.vector.tensor_copy(stg[:, h, :], po[:, 0:129])
                    nc.vector.tensor_copy(den[:, h, :], po.rearrange("p (a e) -> p a e", e=65)[:, :, 64])
                rden = dnp.tile([128, H, 2], F32, tag="rden")
                nc.vector.reciprocal(rden, den)
                ob = obp.tile([128, H, 64], BF16, tag="ob")
                tmp = obp.tile([128, H, 64], BF16, tag="obtmp")
                nc.vector.tensor_tensor(ob, stg[:, :, 0:64],
                                        rden[:, :, 0:1].to_broadcast([128, H, 64]), ALU.mult)
                nc.vector.tensor_tensor(tmp, stg[:, :, 65:129],
                                        rden[:, :, 1:2].to_broadcast([128, H, 64]), ALU.mult)
                nc.vector.tensor_tensor(ob, ob, tmp, ALU.add)
                outb_s.append(ob)

            # outT [128, NID, TCH] via PE transpose of out
            outT = otp.tile([128, NID, TCH], BF16, tag="outT")
            for s in range(NSUB):
                obf = outb_s[s].rearrange("p h e -> p (h e)")
                for f in range(NID):
                    pt = pst.tile([128, 128], BF16, tag="pst")
                    nc.tensor.transpose(pt, obf[:, f * 128:(f + 1) * 128], ident)
                    nc.vector.tensor_copy(outT[:, f, s * SUB:(s + 1) * SUB], pt)

            # l1oT [R+1, TCH]: rows 0..R-1 = o_down^T outT (alpha folded), row R = 1 (bias)
            pl = psb.tile([R, TCH], F32, tag="psb")
            for k in range(NID):
                nc.tensor.matmul(pl, od_[:, k, :], outT[:, k, :], start=(k == 0), stop=(k == NID - 1))
            l1o = l1p.tile([R + 1, TCH], BF16, tag="l1o")
            nc.scalar.activation(l1o[0:R, :], pl, AF.Copy)
            nc.vector.memset(l1o[R:R + 1, :], 1.0)

            # y = outT^T Wo + l1oT^T o_up_aug  -> [tokens, QD]
            for s in range(NSUB):
                rows = slice(t * TCH + s * SUB, t * TCH + (s + 1) * SUB)
                ssl = slice(s * SUB, (s + 1) * SUB)
                ys = ysp.tile([128, QD], F32, tag="ys")
                for half in range(2):
                    nsl = slice(half * 512, (half + 1) * 512)
                    py = psb.tile([128, 512], F32, tag="psb")
                    for k in range(NID):
                        nc.tensor.matmul(py, outT[:, k, ssl], wo[:, k, nsl],
                                         start=(k == 0), stop=False)
                    nc.tensor.matmul(py, l1o[:, ssl], o_up[:, nsl], start=False, stop=True)
                    nc.scalar.activation(ys[:, nsl], py, AF.Copy)
                nc.sync.dma_start(out_ap[rows, :], ys)


def build_nc():
    nc = bacc.Bacc("TRN2", target_bir_lowering=False, debug=False)
    d = {name: nc.dram_tensor(name, shape, F32, kind="ExternalInput").ap()
         for name, shape in IN_SPECS}
    out_ap = nc.dram_tensor("out", [N, QD], F32, kind="ExternalOutput").ap()
    with tile.TileContext(nc) as tc:
        _build(tc, nc, d, out_ap)
    nc.compile()
    return nc


def kernel(**inputs):
    nc = build_nc()
    in_maps = []
    for b in range(B):
        m = {}
        for name, shape in IN_SPECS:
            if name == "x":
                m[name] = np.ascontiguousarray(inputs["x"][b], dtype=np.float32)
            elif name == "context":
                m[name] = np.ascontiguousarray(inputs["context"][b], dtype=np.float32)
            else:
                m[name] = np.ascontiguousarray(inputs[name], dtype=np.float32)
        in_maps.append(m)
    res = run_bass_kernel_spmd(nc, in_maps, core_ids=list(range(B)))
    return np.stack([res.results[b]["out"] for b in range(B)], axis=0)
